# revision 18
# baseline (speedup 1.0000x reference)
"""Trainium2 Bass kernel for nn_LogLinearCDE.

Reference computation:
    y0    = W_in @ x0 + b_in                 # (H,)
    flows = 1 + logsigs @ vf_A               # (L, H)
    ys    = y0 * cumprod(flows, axis=0)      # (L, H)
    out   = softmax(W_out @ ys[-1] + b_out)  # (LABELS,)

Only the LAST row of the cumprod is used, so the kernel reduces to a
per-channel product over L of flows, fused with the generating matmul —
the (L, H) intermediates never exist in HBM.

Sharding: H=4096 split across 8 cores (512 channels each, 4 partition
tiles of 128).

Matmul precision/speed: fp32 matmuls stream at 1/4 rate on TRN2, so
logsigs and vf_A are split into bf16 hi+lo parts and stacked as
    rows  0..16 : a_hi   x  v_hi
    row   17    : 1.0    x  1.0          (the "1 +")
    rows 18..34 : a_lo   x  v_hi
    rows 35..51 : a_hi   x  v_lo
giving  1 + a*v  to ~2^-17 relative accuracy at bf16 streaming speed
(K=52 contraction, fp32 PSUM accumulate).

Per h-tile, time chunks are processed in PAIRS: two 512-col matmuls
into one 2-bank (128, 1024) PSUM tile, consumed whole (FD=1024) to
amortize per-op overhead.  The product-over-L is split between VectorE
(running elementwise product; collapsed at the end by a ScalarE Ln with
fused sum) and ScalarE (Ln with fused sum per pair), all in log space;
a single deferred Exp per core recombines all four h-tiles (avoids
Ln<->Exp activation-table reloads).  y0 is folded into the head weights
host-side (the flow product is strictly positive).  Each core emits a
(1, 10) partial logit row; host sums cores, adds b_out, softmax.
"""

import os
import numpy as np

L = 16384
H = 4096
D = 16
C = 17
LABELS = 10
NCORES = 8
HC = H // NCORES          # 512 channels per core
NT = HC // 128            # 4 h-tiles per core
CHUNK = 512
PAIR = 2 * CHUNK          # 1024: two matmuls -> one 2-bank PSUM tile
NPAIR = L // PAIR         # 16 chunk-pairs per h-tile
K = 3 * C + 1             # 52: hi*hi, const, lo*hi, hi*lo rows

# chunk-pair split between ScalarE (ln+sum) and VectorE (running
# product; collapsed by a GPSIMD multiply tree): "S,V" counts.
SPLIT = tuple(int(x) for x in os.environ.get("KERNEL_SPLIT", "8,8").split(","))
# repeat the compute loop inside the NEFF (differential timing harness)
REPEAT = int(os.environ.get("KERNEL_REPEAT", "1"))

_CACHE = {}


def _pair_paths():
    """Interleave S/V pairs so both engines get work early."""
    s, v = SPLIT
    assert s + v == NPAIR and s >= 1 and v >= 1
    slots = []
    for cls, cnt in (("S", s), ("V", v)):
        slots += [((i + 0.5) / cnt, cls) for i in range(cnt)]
    return [cls for _, cls in sorted(slots)]


def _build_nc(repeat=None):
    import concourse.bacc as bacc
    import concourse.bass as bass
    import concourse.mybir as mybir
    import concourse.tile as tile

    repeat = REPEAT if repeat is None else repeat
    fp32 = mybir.dt.float32
    bf16 = mybir.dt.bfloat16
    nc = bacc.Bacc(None, target_bir_lowering=False)

    lsT_d = nc.dram_tensor("lsT", [K, L], bf16, kind="ExternalInput")
    vfa_d = nc.dram_tensor("vfa", [K, HC], bf16, kind="ExternalInput")
    wouT_d = nc.dram_tensor("wouT", [128, NT * LABELS], fp32, kind="ExternalInput")
    out_d = nc.dram_tensor("out", [1, LABELS], fp32, kind="ExternalOutput")

    paths = _pair_paths()
    n_s = paths.count("S")

    with tile.TileContext(nc) as tc:
        with (
            tc.tile_pool(name="consts", bufs=1) as consts,
            tc.tile_pool(name="racc_pool", bufs=2) as racc_pool,
            tc.tile_pool(name="work", bufs=2) as work,
            tc.tile_pool(name="small", bufs=2 * NT) as small,
            tc.tile_pool(name="psum", bufs=4, space=bass.MemorySpace.PSUM) as psum,
        ):
            lsT = consts.tile([K, L], bf16)
            vfa = consts.tile([K, HC], bf16)
            wouT = consts.tile([128, NT * LABELS], fp32)

            # small weight tensors first (first matmul needs vfa), then
            # spread the big stream across HWDGE (sync) and SWDGE (gpsimd)
            nc.sync.dma_start(vfa[:], vfa_d[:])
            nc.sync.dma_start(wouT[:], wouT_d[:])
            QL = L // 8
            for q in range(8):
                eng = nc.sync if q % 2 == 0 else nc.gpsimd
                eng.dma_start(lsT[:, q * QL:(q + 1) * QL],
                              lsT_d[:, q * QL:(q + 1) * QL])

            for _rep in range(repeat):
                lnt_all = small.tile([128, NT], fp32, tag="lnt_all")
                gp_all = small.tile([128, NT], fp32, tag="gp_all")
                for j in range(NT):
                    racc = racc_pool.tile([128, PAIR], fp32, tag="racc")
                    lnacc = small.tile([128, max(n_s, 1)], fp32, tag="lnacc")

                    vfirst = True
                    si = 0
                    for p in range(NPAIR):
                        fl = psum.tile([128, PAIR], fp32, tag="fl")
                        for h in range(2):
                            nc.tensor.matmul(
                                fl[:, h * CHUNK:(h + 1) * CHUNK],
                                vfa[:, j * 128:(j + 1) * 128],
                                lsT[:, p * PAIR + h * CHUNK:
                                    p * PAIR + (h + 1) * CHUNK],
                                start=True, stop=True,
                            )
                        if paths[p] == "V":
                            if vfirst:
                                nc.vector.tensor_scalar_mul(racc[:], fl[:], 1.0)
                                vfirst = False
                            else:
                                nc.vector.tensor_mul(racc[:], racc[:], fl[:])
                        else:
                            # ln + fused sum over the pair
                            scratch = work.tile([128, PAIR], fp32, tag="scratch")
                            nc.scalar.activation(
                                scratch[:], fl[:],
                                mybir.ActivationFunctionType.Ln,
                                accum_out=lnacc[:, si:si + 1],
                            )
                            si += 1

                    # collapse the running product with a GPSIMD multiply
                    # tree (idle engine; ScalarE/VectorE stay on the pairs)
                    half = PAIR // 2
                    gtree = work.tile([128, half], fp32, tag="gtree")
                    nc.gpsimd.tensor_mul(gtree[:], racc[:, :half],
                                         racc[:, half:])
                    while half > 1:
                        nh = half // 2
                        nc.gpsimd.tensor_mul(gtree[:, :nh], gtree[:, :nh],
                                             gtree[:, nh:half])
                        half = nh
                    nc.gpsimd.tensor_copy(gp_all[:, j:j + 1], gtree[:, :1])

                    # scalar-path log-sum for this tile
                    nc.vector.reduce_sum(
                        lnt_all[:, j:j + 1], lnacc[:, :n_s],
                        axis=mybir.AxisListType.X,
                    )

                # one Exp for all four h-tiles (avoids act-table thrash),
                # then fold in the vector-path products
                ptot_all = small.tile([128, NT], fp32, tag="ptot_all")
                nc.scalar.activation(
                    ptot_all[:], lnt_all[:], mybir.ActivationFunctionType.Exp,
                )
                nc.vector.tensor_mul(ptot_all[:], ptot_all[:], gp_all[:])
                # partial logits: accumulate ptot_j^T @ wouT_j into (1, 10)
                # (y0 is pre-multiplied into wouT host-side); reuse an fl
                # PSUM slot - all quads are consumed by now
                head_ps = psum.tile([1, LABELS], fp32, tag="fl")
                for j in range(NT):
                    nc.tensor.matmul(
                        head_ps[:],
                        ptot_all[:, j:j + 1],
                        wouT[:, j * LABELS:(j + 1) * LABELS],
                        start=(j == 0), stop=(j == NT - 1),
                    )

            head_sb = small.tile([1, LABELS], fp32, tag="head_sb")
            nc.vector.tensor_copy(head_sb[:], head_ps[:])
            nc.sync.dma_start(out_d[:], head_sb[:])

    nc.finalize()
    return nc


def _prep_in_maps(ts, logsigs, x0, W_in, b_in, vf_A, W_out, b_out):
    import ml_dtypes
    bf = ml_dtypes.bfloat16

    logsigs = np.asarray(logsigs, np.float32)
    x0 = np.asarray(x0, np.float32)
    W_in = np.asarray(W_in, np.float32)
    b_in = np.asarray(b_in, np.float32)
    vf_A = np.asarray(vf_A, np.float32)
    W_out = np.asarray(W_out, np.float32)

    a = logsigs.T                                        # (17, L) f32
    a_hi = a.astype(bf)
    a_lo = (a - a_hi.astype(np.float32)).astype(bf)
    ones = np.ones((1, L), bf)
    # rows: a_hi | 1 | a_lo | a_hi   (pair with v_hi | 1 | v_hi | v_lo)
    lsT = np.ascontiguousarray(
        np.concatenate([a_hi, ones, a_lo, a_hi], axis=0))  # (52, L) bf16

    v = vf_A                                             # (17, H) f32
    v_hi = v.astype(bf)
    v_lo = (v - v_hi.astype(np.float32)).astype(bf)
    vones = np.ones((1, H), bf)
    vfa_full = np.concatenate([v_hi, vones, v_hi, v_lo], axis=0)  # (52, H)

    y0 = (W_in @ x0 + b_in).astype(np.float32)           # (H,)
    # fold y0 into the head weights: logits = sum_h W_out[:,h]*y0[h]*P[h]
    Wy = (W_out * y0[None, :]).astype(np.float32)        # (10, H)

    in_maps = []
    for c in range(NCORES):
        sl = slice(c * HC, (c + 1) * HC)
        vfa = np.ascontiguousarray(vfa_full[:, sl])      # (52, 512) bf16
        wT = Wy[:, sl].T.reshape(NT, 128, LABELS)        # (4, 128, 10)
        wouT = np.ascontiguousarray(
            wT.transpose(1, 0, 2).reshape(128, NT * LABELS))
        in_maps.append({"lsT": lsT, "vfa": vfa, "wouT": wouT})
    return in_maps


LAST_EXEC_NS = None
LAST_RESULTS = None


def kernel(ts, logsigs, x0, W_in, b_in, vf_A, W_out, b_out):
    global LAST_EXEC_NS, LAST_RESULTS
    from concourse.bass_utils import run_bass_kernel_spmd

    if "nc" not in _CACHE:
        _CACHE["nc"] = _build_nc()
    nc = _CACHE["nc"]

    in_maps = _prep_in_maps(ts, logsigs, x0, W_in, b_in, vf_A, W_out, b_out)
    trace = bool(int(os.environ.get("KERNEL_TRACE", "0")))
    res = run_bass_kernel_spmd(nc, in_maps, core_ids=list(range(NCORES)),
                               trace=trace)
    LAST_EXEC_NS = res.exec_time_ns
    LAST_RESULTS = res

    partial = np.zeros(LABELS, np.float64)
    for c in range(NCORES):
        partial += res.results[c]["out"][0].astype(np.float64)
    logits = partial + np.asarray(b_out, np.float64)
    z = logits - logits.max()
    ez = np.exp(z)
    return (ez / ez.sum()).astype(np.float32)
